# revision 30
# baseline (speedup 1.0000x reference)
"""Trainium2 Bass kernel for nn_Encoder_Decoder (GRU encoder/decoder over 4096
proposals, batch=1).

Algorithm notes (validated against the reference in numpy):
 - The reference's attention softmaxes an [N,1] tensor over its size-1 axis, so
   every attention weight is exactly 1.0 and `applied = E.sum(axis=0)` is a
   constant across decoder steps. The per-step attention disappears.
 - Each GRU is solved by Picard (fixed-point) iteration over the whole
   sequence: freeze gh_t = Whh^T h_{t-1} from the previous sweep, batch-compute
   all gates, then the recurrence h_t = z_t*h_{t-1} + (1-z_t)*n_t is linear and
   solved exactly with the DVE affine-scan instruction. ~10x error reduction
   per sweep; 4 sweeps land at ~3e-5 relative error (the fp32r noise floor).
 - Everything on-chip is feature-major ([feature, proposal]), so all matmuls
   use natively-oriented weights as lhsT and never need data transposes.
   The three big inputs are transposed once on the host.

Distribution across the 8 NeuronCores:
 - Launch 1 (SPMD x8): the per-proposal feature fusion (memory-bound, ~75MB of
   input reads) is data-parallel over proposals; core c handles rows
   [512c, 512c+512), producing X^T and GiN^T shards.
 - The GRU chain itself is inherently serial with batch=1 (the sharding hint's
   data-parallel-over-images option does not apply), so launch 2 runs the
   Picard sweeps on a single core after a tiny (4MB) host gather.
"""

import os
import sys
import numpy as np

for _p in ("/opt/trn_rl_repo",):
    if _p not in sys.path:
        sys.path.insert(0, _p)

import concourse.bass as bass            # noqa: E402
import concourse.bacc as bacc            # noqa: E402
import concourse.tile as tile            # noqa: E402
from concourse import mybir              # noqa: E402
from concourse.bass_utils import run_bass_kernel_spmd  # noqa: E402

H = 128
N = 4096
CH = 512
NCH = N // CH
NCORES = 8
ENC_SWEEPS = int(os.environ.get("KERNEL_ENC_SWEEPS", "4"))
DEC_SWEEPS = int(os.environ.get("KERNEL_DEC_SWEEPS", "4"))

F32 = mybir.dt.float32
F32R = mybir.dt.float32r
AF = mybir.ActivationFunctionType
ALU = mybir.AluOpType


# ============================ launch 1: fusion ============================

def build_program_a():
    """Per-core feature fusion over a 512-proposal shard (feature-major)."""
    nc = bacc.Bacc("TRN2", target_bir_lowering=False, debug=False)

    def din(name, shape, dt=F32):
        return nc.dram_tensor(name, list(shape), dt, kind="ExternalInput")

    bfT_d = din("bfT", (1024, CH), F32R)
    bsT_d = din("bsT", (2560, CH), F32R)
    bbT_d = din("bbT", (320, CH), F32R)
    Wap_d = din("W_appear", (1024, H), F32R);  bap_d = din("b_appear", (H,))
    Ws1_d = din("W_s1", (2560, 512), F32R);    bs1_d = din("b_s1", (512,))
    Ws2_d = din("W_s2", (512, H), F32R);       bs2_d = din("b_s2", (H,))
    Wbox_d = din("W_box", (320, H), F32R);     bbox_d = din("b_box", (H,))
    Wall_d = din("W_all", (384, H), F32R);     ball_d = din("b_all", (H,))
    eWihN_d = din("enc_Wih_n", (H, H), F32R)
    ebihN_d = din("enc_bih_n", (H,))
    xt_out = nc.dram_tensor("xt_out", [128, CH], F32R, kind="ExternalOutput")
    gin_out = nc.dram_tensor("gin_out", [128, CH], F32, kind="ExternalOutput")

    with tile.TileContext(nc) as tc:
        with tc.tile_pool(name="pa_w", bufs=1) as paw, \
             tc.tile_pool(name="pa_tr", bufs=1) as patr, \
             tc.tile_pool(name="pa_ps", bufs=6, space="PSUM") as paps:

            # 0:bap 1:bs2 2:bbox 3:ball 4-7:bs1 8:ebih_n
            cols = paw.tile([128, 16], F32, tag="cols", name="cols")

            def load_cols(dram_t, n_cols, at):
                src = dram_t.ap().rearrange("(g p) -> p g", p=128)
                nc.sync.dma_start(out=cols[:, at:at + n_cols], in_=src)
            load_cols(bap_d, 1, 0)
            load_cols(bs2_d, 1, 1)
            load_cols(bbox_d, 1, 2)
            load_cols(ball_d, 1, 3)
            load_cols(bs1_d, 4, 4)
            load_cols(ebihN_d, 1, 8)
            eWihN_sb = paw.tile([128, H], F32R, tag="eWihN", name="eWihN")
            nc.sync.dma_start(out=eWihN_sb[:], in_=eWihN_d.ap())

            Ws1_sb = paw.tile([128, 20 * 512], F32R, tag="Ws1", name="Ws1")
            bs_all = patr.tile([128, 20 * CH], F32R, tag="bs_all", name="bs_all")
            # interleave W_s1 / bs shard groups so the dominant matmul chain can
            # start as soon as the first group of each lands
            for g in range(4):
                if g < 2:
                    nc.sync.dma_start(
                        out=Ws1_sb[:, g * 10 * 512:(g + 1) * 10 * 512]
                            .rearrange("p (k m) -> p k m", m=512),
                        in_=Ws1_d.ap()[g * 10 * 128:(g + 1) * 10 * 128, :]
                            .rearrange("(k p) m -> p k m", p=128))
                nc.sync.dma_start(
                    out=bs_all[:, g * 5 * CH:(g + 1) * 5 * CH]
                        .rearrange("p (k c) -> p k c", c=CH),
                    in_=bsT_d.ap()[g * 5 * 128:(g + 1) * 5 * 128, :]
                        .rearrange("(k p) c -> p k c", p=128))
            Wap_sb = paw.tile([128, 8 * H], F32R, tag="Wap", name="Wap")
            nc.sync.dma_start(out=Wap_sb[:].rearrange("p (k m) -> p k m", m=H),
                              in_=Wap_d.ap().rearrange("(k p) m -> p k m", p=128))
            Ws2_sb = paw.tile([128, 4 * H], F32R, tag="Ws2", name="Ws2")
            nc.sync.dma_start(out=Ws2_sb[:].rearrange("p (k m) -> p k m", m=H),
                              in_=Ws2_d.ap().rearrange("(k p) m -> p k m", p=128))
            bb_splits = [(0, 128), (128, 256), (256, 320)]
            Wbox_sb = paw.tile([128, 3 * H], F32R, tag="Wbox", name="Wbox")
            for k, (a, b) in enumerate(bb_splits):
                nc.sync.dma_start(out=Wbox_sb[0:b - a, k * H:(k + 1) * H],
                                  in_=Wbox_d.ap()[a:b, :])
            Wall_sb = paw.tile([128, 3 * H], F32R, tag="Wall", name="Wall")
            for k in range(3):
                nc.sync.dma_start(out=Wall_sb[:, k * H:(k + 1) * H],
                                  in_=Wall_d.ap()[k * 128:(k + 1) * 128, :])

            XT = patr.tile([128, CH], F32R, tag="XT", name="XT")
            bf_all = patr.tile([128, 8 * CH], F32R, tag="bf_all", name="bf_all")
            for g in range(2):
                nc.sync.dma_start(
                    out=bf_all[:, g * 4 * CH:(g + 1) * 4 * CH]
                        .rearrange("p (k c) -> p k c", c=CH),
                    in_=bfT_d.ap()[g * 4 * 128:(g + 1) * 4 * 128, :]
                        .rearrange("(k p) c -> p k c", p=128))
            # --- t1 = bs @ W_s1 + b_s1 (dominant chain: run first) ---
            pt1 = [paps.tile([128, CH], F32, tag="pA", name=f"pt1_{m}") for m in range(4)]
            for k in range(20):
                for m in range(4):
                    nc.tensor.matmul(pt1[m][:],
                                     Ws1_sb[:, k * 512 + m * 128:k * 512 + (m + 1) * 128],
                                     bs_all[:, k * CH:(k + 1) * CH],
                                     start=(k == 0), stop=(k == 19))
            # --- ef = relu(bf @ W_appear + b) ---
            pef = paps.tile([128, CH], F32, tag="pA", name="pef")
            for k in range(8):
                nc.tensor.matmul(pef[:], Wap_sb[:, k * H:(k + 1) * H],
                                 bf_all[:, k * CH:(k + 1) * CH],
                                 start=(k == 0), stop=(k == 7))
            efT = patr.tile([128, CH], F32R, tag="efT", name="efT")
            nc.scalar.activation(efT[:], pef[:], AF.Relu, bias=cols[:, 0:1], scale=1.0)
            t1T = [patr.tile([128, CH], F32R, tag=f"t1T{m}", name=f"t1T{m}", bufs=2)
                   for m in range(4)]
            for m in range(4):
                nc.scalar.activation(t1T[m][:], pt1[m][:], AF.Identity,
                                     bias=cols[:, 4 + m:5 + m], scale=1.0)
            # --- es = relu(t1 @ W_s2 + b_s2) ---
            pes = paps.tile([128, CH], F32, tag="pA", name="pes")
            for k in range(4):
                nc.tensor.matmul(pes[:], Ws2_sb[:, k * H:(k + 1) * H], t1T[k][:],
                                 start=(k == 0), stop=(k == 3))
            esT = patr.tile([128, CH], F32R, tag="esT", name="esT")
            nc.scalar.activation(esT[:], pes[:], AF.Relu, bias=cols[:, 1:2], scale=1.0)
            # --- eb = relu(bb @ W_box + b_box) ---
            bb_all = patr.tile([128, 3 * CH], F32R, tag="bb_all", name="bb_all")
            nc.sync.dma_start(out=bb_all[:, 0:2 * CH].rearrange("p (k c) -> p k c", c=CH),
                              in_=bbT_d.ap()[0:256, :].rearrange("(k p) c -> p k c", p=128))
            nc.sync.dma_start(out=bb_all[0:64, 2 * CH:3 * CH], in_=bbT_d.ap()[256:320, :])
            peb = paps.tile([128, CH], F32, tag="pA", name="peb")
            for k, (a, b) in enumerate(bb_splits):
                nc.tensor.matmul(peb[:], Wbox_sb[0:b - a, k * H:(k + 1) * H],
                                 bb_all[0:b - a, k * CH:(k + 1) * CH],
                                 start=(k == 0), stop=(k == 2))
            ebT = patr.tile([128, CH], F32R, tag="ebT", name="ebT")
            nc.scalar.activation(ebT[:], peb[:], AF.Relu, bias=cols[:, 2:3], scale=1.0)
            # --- X = relu([ef|es|eb] @ W_all + b_all) ---
            pX = paps.tile([128, CH], F32, tag="pA", name="pX")
            for k, src in enumerate((efT, esT, ebT)):
                nc.tensor.matmul(pX[:], Wall_sb[:, k * H:(k + 1) * H], src[:],
                                 start=(k == 0), stop=(k == 2))
            nc.scalar.activation(XT[:], pX[:], AF.Relu, bias=cols[:, 3:4], scale=1.0)
            # --- GiN_enc = Wih_n^T X + bih_n ---
            pG = paps.tile([128, CH], F32, tag="pA", name="pG")
            nc.tensor.matmul(pG[:], eWihN_sb[:], XT[:], start=True, stop=True)
            gin = patr.tile([128, CH], F32, tag="gin", name="gin")
            nc.scalar.activation(gin[:], pG[:], AF.Identity, bias=cols[:, 8:9], scale=1.0)

            nc.sync.dma_start(out=xt_out.ap(), in_=XT[:])
            nc.sync.dma_start(out=gin_out.ap(), in_=gin[:])

    nc.compile()
    return nc


# ============================ launch 2: GRUs ============================

def _gru_sweeps(nc, pbps, pbtr, sweeps, xt, gint, wih_sb, whh_sb,
                brz_rows, bhhn_row, ones_row, hbuf, rz_all, w_all,
                zero_bias):
    """Picard sweeps over hbuf (hbuf[:,0] = h_init, cols 1..N = h-sequence).

    brz_rows: [1, 256] f32r row vector: [bih_r+bhh_r | bih_z+bhh_z].
    rz_all:   [128, 2N]; chunk c holds sigmoid(r) at [c*2CH, c*2CH+CH) and
              sigmoid(z) at [c*2CH+CH, (c+1)*2CH).
    zero_bias: when the GRU biases are all zero (true for this model's inputs)
    the bias psum-matmuls and the sweep-0 r*gh_n term (gh==0) drop out.
    """
    for _s in range(sweeps):
        use_h = _s > 0  # sweep 0's h-sequence guess is all-zeros: skip Whh matmuls
        for c in range(NCH):
            cs, ce = c * CH, (c + 1) * CH
            hprev = hbuf[:, cs:ce]
            # r and z gates share one 2-bank psum so a single sigmoid covers both
            prz = pbps.tile([128, 2 * CH], F32, tag="pr", name="prz")
            for g, half in ((0, prz[:, 0:CH]), (1, prz[:, CH:2 * CH])):
                if use_h:
                    nc.tensor.matmul(half, whh_sb[:, g * H:(g + 1) * H], hprev,
                                     start=True, stop=False)
                nc.tensor.matmul(half, wih_sb[:, g * H:(g + 1) * H], xt[:, cs:ce],
                                 start=not use_h, stop=zero_bias)
                if not zero_bias:
                    nc.tensor.matmul(half, brz_rows[0:1, g * H:(g + 1) * H], ones_row,
                                     start=False, stop=True)
            rz = rz_all[:, c * 2 * CH:(c + 1) * 2 * CH]
            nc.scalar.activation(rz, prz[:], AF.Sigmoid)
            r_t = rz_all[:, c * 2 * CH:c * 2 * CH + CH]
            z_t = rz_all[:, c * 2 * CH + CH:(c + 1) * 2 * CH]
            # n gate: gh_n (+bhh_n), then tanh(gi_n + r*gh_n).
            # nin tiles span chunk PAIRS so tanh runs on [128, 2*CH].
            if c % 2 == 0:
                nin = pbtr.tile([128, 2 * CH], F32, tag="nin", name="nin")
                n_t = pbtr.tile([128, 2 * CH], F32, tag="n", name="n")
            nin_h = nin[:, (c % 2) * CH:(c % 2 + 1) * CH]
            if use_h or not zero_bias:
                pn = pbps.tile([128, CH], F32, tag="pn", name="pn")
                if use_h:
                    nc.tensor.matmul(pn[:], whh_sb[:, 2 * H:3 * H], hprev,
                                     start=True, stop=zero_bias)
                if not zero_bias:
                    nc.tensor.matmul(pn[:], bhhn_row, ones_row, start=not use_h, stop=True)
                t1 = pbtr.tile([128, CH], F32, tag="t1", name="t1")
                nc.vector.tensor_tensor(t1[:], pn[:], r_t, op=ALU.mult)
                # SBUF-only ops go to the otherwise-idle GpSimd engine
                nc.gpsimd.tensor_tensor(nin_h, t1[:], gint[:, cs:ce], op=ALU.add)
            else:
                # sweep 0 with zero biases: gh_n == 0, so nin == gi_n
                nc.gpsimd.tensor_copy(nin_h, gint[:, cs:ce])
            if c % 2 == 1:
                nc.scalar.activation(n_t[:], nin[:], AF.Tanh)
                for cc in (c - 1, c):
                    zslice = rz_all[:, cc * 2 * CH + CH:(cc + 1) * 2 * CH]
                    nc.vector.scalar_tensor_tensor(
                        out=w_all[:, cc * CH:(cc + 1) * CH], in0=zslice,
                        scalar=1.0, in1=n_t[:, (cc % 2) * CH:(cc % 2 + 1) * CH],
                        op0=ALU.subtract, op1=ALU.mult)
        # linear recurrence h_t = z_t*h_{t-1} - w_neg_t via chunked affine scan
        for c in range(NCH):
            cs, ce = c * CH, (c + 1) * CH
            nc.vector.tensor_tensor_scan(
                out=hbuf[:, cs + 1:ce + 1],
                data0=rz_all[:, c * 2 * CH + CH:(c + 1) * 2 * CH],
                data1=w_all[:, cs:ce],
                initial=hbuf[:, cs:cs + 1],
                op0=ALU.mult, op1=ALU.subtract)


def build_program_b(zero_bias=True):
    nc = bacc.Bacc("TRN2", target_bir_lowering=False, debug=False)

    def din(name, shape, dt=F32):
        return nc.dram_tensor(name, list(shape), dt, kind="ExternalInput")

    XT_d = din("XT", (128, N), F32R)
    GiNT_d = din("GiNT", (128, N))
    eWih_d = din("enc_Wih", (H, 3 * H), F32R); eWhh_d = din("enc_Whh", (H, 3 * H), F32R)
    ebih_d = din("enc_bih", (3 * H,));   ebhh_d = din("enc_bhh", (3 * H,))
    dWih_d = din("dec_Wih", (H, 3 * H), F32R); dWhh_d = din("dec_Whh", (H, 3 * H), F32R)
    dbih_d = din("dec_bih", (3 * H,));   dbhh_d = din("dec_bhh", (3 * H,))
    Wcomb_d = din("W_comb", (2 * H, H), F32R); bcomb_d = din("b_comb", (H,))
    Wout_d = din("W_out", (H, 1), F32R);       bout_d = din("b_out", (1,))
    ones_d = din("ones_row", (1, CH), F32R)
    zcol_d = din("zeros_col", (128, 1), F32R)
    out_d = nc.dram_tensor("out", [N], F32, kind="ExternalOutput")

    with tile.TileContext(nc) as tc:
        with tc.tile_pool(name="persist", bufs=1) as persist, \
             tc.tile_pool(name="gruw", bufs=1) as gruw:

            # ---- persistent big tiles ----
            XinT = persist.tile([128, N], F32R, tag="XinT", name="XinT")   # X^T, later Din^T
            GiNT = persist.tile([128, N], F32, tag="GiNT", name="GiNT")
            rz_all = persist.tile([128, 2 * N], F32, tag="rz_all", name="rz_all")
            w_all = persist.tile([128, N], F32, tag="w_all", name="w_all")
            Hbuf = persist.tile([128, N + 1], F32R, tag="Hbuf", name="Hbuf")

            nc.sync.dma_start(out=XinT[:], in_=XT_d.ap())
            nc.sync.dma_start(out=GiNT[:], in_=GiNT_d.ap())

            # ---- packed per-partition constant columns [128, 32] ----
            # 4:bcomb 9-11:ebih 12-14:ebhh 15-17:dbih 18-20:dbhh
            # 21:e_brz_r 22:e_brz_z 23:d_brz_r 24:d_brz_z
            cols = gruw.tile([128, 32], F32, tag="cols", name="cols")

            def load_cols(dram_t, n_cols, at):
                src = dram_t.ap().rearrange("(g p) -> p g", p=128)
                nc.sync.dma_start(out=cols[:, at:at + n_cols], in_=src)
            load_cols(bcomb_d, 1, 4)
            load_cols(ebih_d, 3, 9)
            load_cols(ebhh_d, 3, 12)
            load_cols(dbih_d, 3, 15)
            load_cols(dbhh_d, 3, 18)
            # gate-bias ROW vectors (bih+bhh for r,z), f32r, for psum bias-matmuls
            brow_f32 = gruw.tile([1, 3 * 256], F32, tag="brow_f32", name="brow_f32")
            nc.sync.dma_start(out=brow_f32[0:1, 0:256],
                              in_=ebih_d.ap()[0:256].rearrange("(o f) -> o f", o=1))
            nc.sync.dma_start(out=brow_f32[0:1, 256:512],
                              in_=ebhh_d.ap()[0:256].rearrange("(o f) -> o f", o=1))
            nc.vector.tensor_tensor(brow_f32[0:1, 512:768], brow_f32[0:1, 0:256],
                                    brow_f32[0:1, 256:512], op=ALU.add)
            e_brz_rows = gruw.tile([1, 256], F32R, tag="e_brz", name="e_brz")
            nc.vector.tensor_copy(e_brz_rows[:], brow_f32[0:1, 512:768])
            nc.sync.dma_start(out=brow_f32[0:1, 0:256],
                              in_=dbih_d.ap()[0:256].rearrange("(o f) -> o f", o=1))
            nc.sync.dma_start(out=brow_f32[0:1, 256:512],
                              in_=dbhh_d.ap()[0:256].rearrange("(o f) -> o f", o=1))
            nc.vector.tensor_tensor(brow_f32[0:1, 512:768], brow_f32[0:1, 0:256],
                                    brow_f32[0:1, 256:512], op=ALU.add)
            d_brz_rows = gruw.tile([1, 256], F32R, tag="d_brz", name="d_brz")
            nc.vector.tensor_copy(d_brz_rows[:], brow_f32[0:1, 512:768])

            # ---- single-partition rows ----
            ones_row = gruw.tile([1, CH], F32R, tag="ones", name="ones")
            nc.sync.dma_start(out=ones_row[:], in_=ones_d.ap())
            rows = gruw.tile([1, 2 * H], F32R, tag="rows", name="rows")
            bout_sb = gruw.tile([1, 1], F32, tag="bout", name="bout")
            nc.sync.dma_start(out=rows[0:1, 0:H],
                              in_=ebhh_d.ap()[2 * H:3 * H].rearrange("(o f) -> o f", o=1).bitcast(F32R))
            nc.sync.dma_start(out=rows[0:1, H:2 * H],
                              in_=dbhh_d.ap()[2 * H:3 * H].rearrange("(o f) -> o f", o=1).bitcast(F32R))
            nc.sync.dma_start(out=bout_sb[:],
                              in_=bout_d.ap().rearrange("(o f) -> o f", o=1))

            # ---- GRU / head weights packed ----
            gw = gruw.tile([128, 4 * 3 * H + 2 * H + 1], F32R, tag="gw", name="gw")
            eWih_sb = gw[:, 0:384]
            eWhh_sb = gw[:, 384:768]
            dWih_sb = gw[:, 768:1152]
            dWhh_sb = gw[:, 1152:1536]
            Wc1_sb = gw[:, 1536:1664]
            Wc2_sb = gw[:, 1664:1792]
            Wout_sb = gw[:, 1792:1793]
            nc.sync.dma_start(out=eWih_sb, in_=eWih_d.ap())
            nc.sync.dma_start(out=eWhh_sb, in_=eWhh_d.ap())
            nc.sync.dma_start(out=dWih_sb, in_=dWih_d.ap())
            nc.sync.dma_start(out=dWhh_sb, in_=dWhh_d.ap())
            nc.sync.dma_start(out=Wc1_sb, in_=Wcomb_d.ap()[0:H, :])
            nc.sync.dma_start(out=Wc2_sb, in_=Wcomb_d.ap()[H:2 * H, :])
            nc.sync.dma_start(out=Wout_sb, in_=Wout_d.ap())

            with tc.tile_pool(name="pb_ps", bufs=2, space="PSUM") as pbps, \
                 tc.tile_pool(name="pb_tr", bufs=3) as pbtr:

                # encoder (initial h = zeros; sweep-0 Picard guess is implicit zeros)
                nc.sync.dma_start(out=Hbuf[:, 0:1], in_=zcol_d.ap())
                _gru_sweeps(nc, pbps, pbtr, ENC_SWEEPS, XinT, GiNT,
                            eWih_sb, eWhh_sb, e_brz_rows[:],
                            rows[0:1, 0:H], ones_row[:], Hbuf, rz_all, w_all,
                            zero_bias)

                # applied = E.sum(0): 8 partial reductions (overlap the last scans)
                s_part = pbtr.tile([128, 9], F32, tag="s_part", name="s_part")
                for c in range(NCH):
                    nc.vector.tensor_reduce(s_part[:, c:c + 1], Hbuf[:, c * CH + 1:(c + 1) * CH + 1],
                                            axis=mybir.AxisListType.X, op=ALU.add)
                s_col = pbtr.tile([128, 1], F32, tag="s_col", name="s_col")
                nc.vector.tensor_reduce(s_col[:], s_part[:, 0:8], axis=mybir.AxisListType.X, op=ALU.add)
                henc = pbtr.tile([128, 1], F32R, tag="henc", name="henc")
                nc.vector.tensor_copy(henc[:], Hbuf[:, N:N + 1])
                pc = pbps.tile([128, 1], F32, tag="pn", name="pc")
                nc.tensor.matmul(pc[:], Wc2_sb.bitcast(F32), s_col[:], start=True, stop=True)
                c_col = pbtr.tile([128, 1], F32, tag="c_col", name="c_col")
                nc.scalar.activation(c_col[:], pc[:], AF.Identity, bias=cols[:, 4:5], scale=1.0)

                # Din = relu(W_comb[:H]^T X + c)  (overwrites XinT), then dec GiN
                for c in range(NCH // 2):
                    cs = c * 2 * CH
                    pD = pbps.tile([128, 2 * CH], F32, tag="pr", name="pD")
                    nc.tensor.matmul(pD[:, 0:CH], Wc1_sb, XinT[:, cs:cs + CH], start=True, stop=True)
                    nc.tensor.matmul(pD[:, CH:2 * CH], Wc1_sb, XinT[:, cs + CH:cs + 2 * CH],
                                     start=True, stop=True)
                    nc.scalar.activation(XinT[:, cs:cs + 2 * CH], pD[:], AF.Relu,
                                         bias=c_col[:, 0:1], scale=1.0)
                for c in range(NCH // 2):
                    cs = c * 2 * CH
                    pG = pbps.tile([128, 2 * CH], F32, tag="pr", name="pG2")
                    nc.tensor.matmul(pG[:, 0:CH], dWih_sb[:, 2 * H:3 * H], XinT[:, cs:cs + CH],
                                     start=True, stop=True)
                    nc.tensor.matmul(pG[:, CH:2 * CH], dWih_sb[:, 2 * H:3 * H],
                                     XinT[:, cs + CH:cs + 2 * CH], start=True, stop=True)
                    nc.scalar.activation(GiNT[:, cs:cs + 2 * CH], pG[:], AF.Identity,
                                         bias=cols[:, 17:18], scale=1.0)

                # decoder (initial h = h_enc; sweep-0 Picard guess is implicit zeros)
                nc.vector.tensor_copy(Hbuf[:, 0:1], henc[:])
                _gru_sweeps(nc, pbps, pbtr, DEC_SWEEPS, XinT, GiNT,
                            dWih_sb, dWhh_sb, d_brz_rows[:],
                            rows[0:1, H:2 * H], ones_row[:], Hbuf, rz_all, w_all,
                            zero_bias)

                # output = sigmoid(D @ W_out + b_out), chunk pairs, streamed out
                for c in range(NCH // 2):
                    cs = c * 2 * CH
                    po = pbps.tile([1, 2 * CH], F32, tag="pn", name="po")
                    nc.tensor.matmul(po[:, 0:CH], Wout_sb, Hbuf[:, cs + 1:cs + CH + 1],
                                     start=True, stop=True)
                    nc.tensor.matmul(po[:, CH:2 * CH], Wout_sb, Hbuf[:, cs + CH + 1:cs + 2 * CH + 1],
                                     start=True, stop=True)
                    o_t = pbtr.tile([1, 2 * CH], F32, tag="o_t", name="o_t")
                    nc.scalar.activation(o_t[:], po[:], AF.Sigmoid,
                                         bias=bout_sb[0:1, 0:1], scale=1.0)
                    nc.sync.dma_start(out=out_d.ap()[cs:cs + 2 * CH].rearrange("(o n) -> o n", o=1),
                                      in_=o_t[:])

    nc.compile()
    return nc


_NC_A = None
_NC_B = {}


def _get_programs(zero_bias):
    global _NC_A
    if _NC_A is None:
        _NC_A = build_program_a()
    if zero_bias not in _NC_B:
        _NC_B[zero_bias] = build_program_b(zero_bias)
    return _NC_A, _NC_B[zero_bias]


_LAST_EXEC_NS = None


def kernel(**inputs):
    global _LAST_EXEC_NS
    zero_bias = not any(
        np.any(np.asarray(inputs[k]) != 0)
        for k in ("enc_bih", "enc_bhh", "dec_bih", "dec_bhh"))
    nc_a, nc_b = _get_programs(zero_bias)
    f32c = lambda a: np.ascontiguousarray(np.asarray(a), dtype=np.float32)

    bfT = f32c(np.asarray(inputs["boxes_feature"])[0].T)   # [1024, N]
    bsT = f32c(np.asarray(inputs["boxes_box_score"])[0].T)  # [2560, N]
    bbT = f32c(np.asarray(inputs["boxes_box"])[0].T)        # [320, N]
    weights_a = {
        "W_appear": f32c(inputs["W_appear"]), "b_appear": f32c(inputs["b_appear"]),
        "W_s1": f32c(inputs["W_s1"]), "b_s1": f32c(inputs["b_s1"]),
        "W_s2": f32c(inputs["W_s2"]), "b_s2": f32c(inputs["b_s2"]),
        "W_box": f32c(inputs["W_box"]), "b_box": f32c(inputs["b_box"]),
        "W_all": f32c(inputs["W_all"]), "b_all": f32c(inputs["b_all"]),
        "enc_Wih_n": f32c(np.asarray(inputs["enc_Wih"])[:, 2 * H:3 * H]),
        "enc_bih_n": f32c(np.asarray(inputs["enc_bih"])[2 * H:3 * H]),
    }
    in_maps_a = []
    for c in range(NCORES):
        cs, ce = c * CH, (c + 1) * CH
        in_maps_a.append({
            "bfT": f32c(bfT[:, cs:ce]),
            "bsT": f32c(bsT[:, cs:ce]),
            "bbT": f32c(bbT[:, cs:ce]),
            **weights_a,
        })
    res_a = run_bass_kernel_spmd(nc_a, in_maps_a, list(range(NCORES)))
    XT = np.concatenate([np.asarray(r["xt_out"]) for r in res_a.results], axis=1)
    GiNT = np.concatenate([np.asarray(r["gin_out"]) for r in res_a.results], axis=1)

    in_map_b = {
        "XT": np.ascontiguousarray(XT), "GiNT": np.ascontiguousarray(GiNT),
        "enc_Wih": f32c(inputs["enc_Wih"]), "enc_Whh": f32c(inputs["enc_Whh"]),
        "enc_bih": f32c(inputs["enc_bih"]), "enc_bhh": f32c(inputs["enc_bhh"]),
        "dec_Wih": f32c(inputs["dec_Wih"]), "dec_Whh": f32c(inputs["dec_Whh"]),
        "dec_bih": f32c(inputs["dec_bih"]), "dec_bhh": f32c(inputs["dec_bhh"]),
        "W_comb": f32c(inputs["W_comb"]), "b_comb": f32c(inputs["b_comb"]),
        "W_out": f32c(inputs["W_out"]), "b_out": f32c(inputs["b_out"]),
        "ones_row": np.ones((1, CH), np.float32),
        "zeros_col": np.zeros((128, 1), np.float32),
    }
    res_b = run_bass_kernel_spmd(nc_b, [in_map_b], [0])
    out = np.asarray(res_b.results[0]["out"], dtype=np.float32).reshape(N, 1)
    labels = np.asarray(inputs["boxes_label"], dtype=np.float32)[0, :N]
    weights = np.asarray(inputs["boxes_weight"], dtype=np.float32)[0, :N]
    return out, labels, weights


# revision 32
# speedup vs baseline: 1.0295x; 1.0295x over previous
"""Trainium2 Bass kernel for nn_Encoder_Decoder (GRU encoder/decoder over 4096
proposals, batch=1).

Algorithm notes (validated against the reference in numpy):
 - The reference's attention softmaxes an [N,1] tensor over its size-1 axis, so
   every attention weight is exactly 1.0 and `applied = E.sum(axis=0)` is a
   constant across decoder steps. The per-step attention disappears.
 - Each GRU is solved by Picard (fixed-point) iteration over the whole
   sequence: freeze gh_t = Whh^T h_{t-1} from the previous sweep, batch-compute
   all gates, then the recurrence h_t = z_t*h_{t-1} + (1-z_t)*n_t is linear and
   solved exactly with the DVE affine-scan instruction. ~10x error reduction
   per sweep; 4 sweeps land at ~3e-5 relative error (the fp32r noise floor).
 - Everything on-chip is feature-major ([feature, proposal]), so all matmuls
   use natively-oriented weights as lhsT and never need data transposes.
   The three big inputs are transposed once on the host.

Distribution across the 8 NeuronCores:
 - Launch 1 (SPMD x8): the per-proposal feature fusion (memory-bound, ~75MB of
   input reads) is data-parallel over proposals; core c handles rows
   [512c, 512c+512), producing X^T and GiN^T shards.
 - The GRU chain itself is inherently serial with batch=1 (the sharding hint's
   data-parallel-over-images option does not apply), so launch 2 runs the
   Picard sweeps on a single core after a tiny (4MB) host gather.
"""

import os
import sys
import numpy as np

for _p in ("/opt/trn_rl_repo",):
    if _p not in sys.path:
        sys.path.insert(0, _p)

import concourse.bass as bass            # noqa: E402
import concourse.bacc as bacc            # noqa: E402
import concourse.tile as tile            # noqa: E402
from concourse import mybir              # noqa: E402
from concourse.bass_utils import run_bass_kernel_spmd  # noqa: E402

H = 128
N = 4096
CH = 512
NCH = N // CH
NCORES = 8
ENC_SWEEPS = int(os.environ.get("KERNEL_ENC_SWEEPS", "4"))
DEC_SWEEPS = int(os.environ.get("KERNEL_DEC_SWEEPS", "4"))

F32 = mybir.dt.float32
F32R = mybir.dt.float32r
AF = mybir.ActivationFunctionType
ALU = mybir.AluOpType


# ============================ launch 1: fusion ============================

def build_program_a():
    """Per-core feature fusion over a 512-proposal shard (feature-major)."""
    nc = bacc.Bacc("TRN2", target_bir_lowering=False, debug=False)

    def din(name, shape, dt=F32):
        return nc.dram_tensor(name, list(shape), dt, kind="ExternalInput")

    bfT_d = din("bfT", (1024, CH), F32R)
    bsT_d = din("bsT", (2560, CH), F32R)
    bbT_d = din("bbT", (320, CH), F32R)
    Wap_d = din("W_appear", (1024, H), F32R);  bap_d = din("b_appear", (H,))
    Ws1_d = din("W_s1", (2560, 512), F32R);    bs1_d = din("b_s1", (512,))
    Ws2_d = din("W_s2", (512, H), F32R);       bs2_d = din("b_s2", (H,))
    Wbox_d = din("W_box", (320, H), F32R);     bbox_d = din("b_box", (H,))
    Wall_d = din("W_all", (384, H), F32R);     ball_d = din("b_all", (H,))
    eWihN_d = din("enc_Wih_n", (H, H), F32R)
    ebihN_d = din("enc_bih_n", (H,))
    xt_out = nc.dram_tensor("xt_out", [128, CH], F32R, kind="ExternalOutput")
    gin_out = nc.dram_tensor("gin_out", [128, CH], F32, kind="ExternalOutput")

    with tile.TileContext(nc) as tc:
        with tc.tile_pool(name="pa_w", bufs=1) as paw, \
             tc.tile_pool(name="pa_tr", bufs=1) as patr, \
             tc.tile_pool(name="pa_ps", bufs=6, space="PSUM") as paps:

            # 0:bap 1:bs2 2:bbox 3:ball 4-7:bs1 8:ebih_n
            cols = paw.tile([128, 16], F32, tag="cols", name="cols")

            def load_cols(dram_t, n_cols, at):
                src = dram_t.ap().rearrange("(g p) -> p g", p=128)
                nc.sync.dma_start(out=cols[:, at:at + n_cols], in_=src)
            load_cols(bap_d, 1, 0)
            load_cols(bs2_d, 1, 1)
            load_cols(bbox_d, 1, 2)
            load_cols(ball_d, 1, 3)
            load_cols(bs1_d, 4, 4)
            load_cols(ebihN_d, 1, 8)
            eWihN_sb = paw.tile([128, H], F32R, tag="eWihN", name="eWihN")
            nc.sync.dma_start(out=eWihN_sb[:], in_=eWihN_d.ap())

            Ws1_sb = paw.tile([128, 20 * 512], F32R, tag="Ws1", name="Ws1")
            bs_all = patr.tile([128, 20 * CH], F32R, tag="bs_all", name="bs_all")
            # interleave W_s1 / bs shard k-slice groups, smallest first, so the
            # dominant matmul chain starts as soon as the first pair lands
            bounds = [0, 2, 6, 12, 20]
            for g in range(4):
                a, b = bounds[g], bounds[g + 1]
                nc.sync.dma_start(
                    out=Ws1_sb[:, a * 512:b * 512].rearrange("p (k m) -> p k m", m=512),
                    in_=Ws1_d.ap()[a * 128:b * 128, :].rearrange("(k p) m -> p k m", p=128))
                nc.sync.dma_start(
                    out=bs_all[:, a * CH:b * CH].rearrange("p (k c) -> p k c", c=CH),
                    in_=bsT_d.ap()[a * 128:b * 128, :].rearrange("(k p) c -> p k c", p=128))
            Wap_sb = paw.tile([128, 8 * H], F32R, tag="Wap", name="Wap")
            nc.sync.dma_start(out=Wap_sb[:].rearrange("p (k m) -> p k m", m=H),
                              in_=Wap_d.ap().rearrange("(k p) m -> p k m", p=128))
            Ws2_sb = paw.tile([128, 4 * H], F32R, tag="Ws2", name="Ws2")
            nc.sync.dma_start(out=Ws2_sb[:].rearrange("p (k m) -> p k m", m=H),
                              in_=Ws2_d.ap().rearrange("(k p) m -> p k m", p=128))
            bb_splits = [(0, 128), (128, 256), (256, 320)]
            Wbox_sb = paw.tile([128, 3 * H], F32R, tag="Wbox", name="Wbox")
            for k, (a, b) in enumerate(bb_splits):
                nc.sync.dma_start(out=Wbox_sb[0:b - a, k * H:(k + 1) * H],
                                  in_=Wbox_d.ap()[a:b, :])
            Wall_sb = paw.tile([128, 3 * H], F32R, tag="Wall", name="Wall")
            for k in range(3):
                nc.sync.dma_start(out=Wall_sb[:, k * H:(k + 1) * H],
                                  in_=Wall_d.ap()[k * 128:(k + 1) * 128, :])

            XT = patr.tile([128, CH], F32R, tag="XT", name="XT")
            bf_all = patr.tile([128, 8 * CH], F32R, tag="bf_all", name="bf_all")
            for g in range(2):
                nc.sync.dma_start(
                    out=bf_all[:, g * 4 * CH:(g + 1) * 4 * CH]
                        .rearrange("p (k c) -> p k c", c=CH),
                    in_=bfT_d.ap()[g * 4 * 128:(g + 1) * 4 * 128, :]
                        .rearrange("(k p) c -> p k c", p=128))
            # --- t1 = bs @ W_s1 + b_s1 (dominant chain: run first) ---
            pt1 = [paps.tile([128, CH], F32, tag="pA", name=f"pt1_{m}") for m in range(4)]
            for k in range(20):
                for m in range(4):
                    nc.tensor.matmul(pt1[m][:],
                                     Ws1_sb[:, k * 512 + m * 128:k * 512 + (m + 1) * 128],
                                     bs_all[:, k * CH:(k + 1) * CH],
                                     start=(k == 0), stop=(k == 19))
            # --- ef = relu(bf @ W_appear + b) ---
            pef = paps.tile([128, CH], F32, tag="pA", name="pef")
            for k in range(8):
                nc.tensor.matmul(pef[:], Wap_sb[:, k * H:(k + 1) * H],
                                 bf_all[:, k * CH:(k + 1) * CH],
                                 start=(k == 0), stop=(k == 7))
            efT = patr.tile([128, CH], F32R, tag="efT", name="efT")
            nc.scalar.activation(efT[:], pef[:], AF.Relu, bias=cols[:, 0:1], scale=1.0)
            t1T = [patr.tile([128, CH], F32R, tag=f"t1T{m}", name=f"t1T{m}", bufs=2)
                   for m in range(4)]
            for m in range(4):
                nc.scalar.activation(t1T[m][:], pt1[m][:], AF.Identity,
                                     bias=cols[:, 4 + m:5 + m], scale=1.0)
            # --- es = relu(t1 @ W_s2 + b_s2) ---
            pes = paps.tile([128, CH], F32, tag="pA", name="pes")
            for k in range(4):
                nc.tensor.matmul(pes[:], Ws2_sb[:, k * H:(k + 1) * H], t1T[k][:],
                                 start=(k == 0), stop=(k == 3))
            esT = patr.tile([128, CH], F32R, tag="esT", name="esT")
            nc.scalar.activation(esT[:], pes[:], AF.Relu, bias=cols[:, 1:2], scale=1.0)
            # --- eb = relu(bb @ W_box + b_box) ---
            bb_all = patr.tile([128, 3 * CH], F32R, tag="bb_all", name="bb_all")
            nc.sync.dma_start(out=bb_all[:, 0:2 * CH].rearrange("p (k c) -> p k c", c=CH),
                              in_=bbT_d.ap()[0:256, :].rearrange("(k p) c -> p k c", p=128))
            nc.sync.dma_start(out=bb_all[0:64, 2 * CH:3 * CH], in_=bbT_d.ap()[256:320, :])
            peb = paps.tile([128, CH], F32, tag="pA", name="peb")
            for k, (a, b) in enumerate(bb_splits):
                nc.tensor.matmul(peb[:], Wbox_sb[0:b - a, k * H:(k + 1) * H],
                                 bb_all[0:b - a, k * CH:(k + 1) * CH],
                                 start=(k == 0), stop=(k == 2))
            ebT = patr.tile([128, CH], F32R, tag="ebT", name="ebT")
            nc.scalar.activation(ebT[:], peb[:], AF.Relu, bias=cols[:, 2:3], scale=1.0)
            # --- X = relu([ef|es|eb] @ W_all + b_all) ---
            pX = paps.tile([128, CH], F32, tag="pA", name="pX")
            for k, src in enumerate((efT, esT, ebT)):
                nc.tensor.matmul(pX[:], Wall_sb[:, k * H:(k + 1) * H], src[:],
                                 start=(k == 0), stop=(k == 2))
            nc.scalar.activation(XT[:], pX[:], AF.Relu, bias=cols[:, 3:4], scale=1.0)
            # --- GiN_enc = Wih_n^T X + bih_n ---
            pG = paps.tile([128, CH], F32, tag="pA", name="pG")
            nc.tensor.matmul(pG[:], eWihN_sb[:], XT[:], start=True, stop=True)
            gin = patr.tile([128, CH], F32, tag="gin", name="gin")
            nc.scalar.activation(gin[:], pG[:], AF.Identity, bias=cols[:, 8:9], scale=1.0)

            nc.sync.dma_start(out=xt_out.ap(), in_=XT[:])
            nc.sync.dma_start(out=gin_out.ap(), in_=gin[:])

    nc.compile()
    return nc


# ============================ launch 2: GRUs ============================

def _gru_sweeps(nc, pbps, pbtr, sweeps, xt, gint, wih_sb, whh_sb,
                brz_rows, bhhn_row, ones_row, hbuf, rz_all, w_all,
                zero_bias):
    """Picard sweeps over hbuf (hbuf[:,0] = h_init, cols 1..N = h-sequence).

    brz_rows: [1, 256] f32r row vector: [bih_r+bhh_r | bih_z+bhh_z].
    rz_all:   [128, 2N]; chunk c holds sigmoid(r) at [c*2CH, c*2CH+CH) and
              sigmoid(z) at [c*2CH+CH, (c+1)*2CH).
    zero_bias: when the GRU biases are all zero (true for this model's inputs)
    the bias psum-matmuls and the sweep-0 r*gh_n term (gh==0) drop out.
    """
    for _s in range(sweeps):
        use_h = _s > 0  # sweep 0's h-sequence guess is all-zeros: skip Whh matmuls
        for c in range(NCH):
            cs, ce = c * CH, (c + 1) * CH
            hprev = hbuf[:, cs:ce]
            # r and z gates share one 2-bank psum so a single sigmoid covers both;
            # sweep 0 with zero biases never uses r (gh==0), so do only z there.
            prz = pbps.tile([128, 2 * CH], F32, tag="pr", name="prz")
            gates = ((0, prz[:, 0:CH]), (1, prz[:, CH:2 * CH]))
            if zero_bias and not use_h:
                gates = gates[1:]
            for g, half in gates:
                if use_h:
                    nc.tensor.matmul(half, whh_sb[:, g * H:(g + 1) * H], hprev,
                                     start=True, stop=False)
                nc.tensor.matmul(half, wih_sb[:, g * H:(g + 1) * H], xt[:, cs:ce],
                                 start=not use_h, stop=zero_bias)
                if not zero_bias:
                    nc.tensor.matmul(half, brz_rows[0:1, g * H:(g + 1) * H], ones_row,
                                     start=False, stop=True)
            if zero_bias and not use_h:
                nc.scalar.activation(rz_all[:, c * 2 * CH + CH:(c + 1) * 2 * CH],
                                     prz[:, CH:2 * CH], AF.Sigmoid)
            else:
                nc.scalar.activation(rz_all[:, c * 2 * CH:(c + 1) * 2 * CH],
                                     prz[:], AF.Sigmoid)
            r_t = rz_all[:, c * 2 * CH:c * 2 * CH + CH]
            z_t = rz_all[:, c * 2 * CH + CH:(c + 1) * 2 * CH]
            # n gate: gh_n (+bhh_n), then tanh(gi_n + r*gh_n).
            # nin tiles span chunk PAIRS so tanh runs on [128, 2*CH].
            if c % 2 == 0:
                nin = pbtr.tile([128, 2 * CH], F32, tag="nin", name="nin")
                n_t = pbtr.tile([128, 2 * CH], F32, tag="n", name="n")
            nin_h = nin[:, (c % 2) * CH:(c % 2 + 1) * CH]
            if use_h or not zero_bias:
                pn = pbps.tile([128, CH], F32, tag="pn", name="pn")
                if use_h:
                    nc.tensor.matmul(pn[:], whh_sb[:, 2 * H:3 * H], hprev,
                                     start=True, stop=zero_bias)
                if not zero_bias:
                    nc.tensor.matmul(pn[:], bhhn_row, ones_row, start=not use_h, stop=True)
                t1 = pbtr.tile([128, CH], F32, tag="t1", name="t1")
                nc.vector.tensor_tensor(t1[:], pn[:], r_t, op=ALU.mult)
                # SBUF-only ops go to the otherwise-idle GpSimd engine
                nc.gpsimd.tensor_tensor(nin_h, t1[:], gint[:, cs:ce], op=ALU.add)
            else:
                # sweep 0 with zero biases: gh_n == 0, so nin == gi_n
                nc.gpsimd.tensor_copy(nin_h, gint[:, cs:ce])
            if c % 2 == 1:
                nc.scalar.activation(n_t[:], nin[:], AF.Tanh)
                for cc in (c - 1, c):
                    zslice = rz_all[:, cc * 2 * CH + CH:(cc + 1) * 2 * CH]
                    nc.vector.scalar_tensor_tensor(
                        out=w_all[:, cc * CH:(cc + 1) * CH], in0=zslice,
                        scalar=1.0, in1=n_t[:, (cc % 2) * CH:(cc % 2 + 1) * CH],
                        op0=ALU.subtract, op1=ALU.mult)
        # linear recurrence h_t = z_t*h_{t-1} - w_neg_t via chunked affine scan
        for c in range(NCH):
            cs, ce = c * CH, (c + 1) * CH
            nc.vector.tensor_tensor_scan(
                out=hbuf[:, cs + 1:ce + 1],
                data0=rz_all[:, c * 2 * CH + CH:(c + 1) * 2 * CH],
                data1=w_all[:, cs:ce],
                initial=hbuf[:, cs:cs + 1],
                op0=ALU.mult, op1=ALU.subtract)


def build_program_b(zero_bias=True):
    nc = bacc.Bacc("TRN2", target_bir_lowering=False, debug=False)

    def din(name, shape, dt=F32):
        return nc.dram_tensor(name, list(shape), dt, kind="ExternalInput")

    XT_d = din("XT", (128, N), F32R)
    GiNT_d = din("GiNT", (128, N))
    eWih_d = din("enc_Wih", (H, 3 * H), F32R); eWhh_d = din("enc_Whh", (H, 3 * H), F32R)
    ebih_d = din("enc_bih", (3 * H,));   ebhh_d = din("enc_bhh", (3 * H,))
    dWih_d = din("dec_Wih", (H, 3 * H), F32R); dWhh_d = din("dec_Whh", (H, 3 * H), F32R)
    dbih_d = din("dec_bih", (3 * H,));   dbhh_d = din("dec_bhh", (3 * H,))
    Wcomb_d = din("W_comb", (2 * H, H), F32R); bcomb_d = din("b_comb", (H,))
    Wout_d = din("W_out", (H, 1), F32R);       bout_d = din("b_out", (1,))
    ones_d = din("ones_row", (1, CH), F32R)
    zcol_d = din("zeros_col", (128, 1), F32R)
    out_d = nc.dram_tensor("out", [N], F32, kind="ExternalOutput")

    with tile.TileContext(nc) as tc:
        with tc.tile_pool(name="persist", bufs=1) as persist, \
             tc.tile_pool(name="gruw", bufs=1) as gruw:

            # ---- persistent big tiles ----
            XinT = persist.tile([128, N], F32R, tag="XinT", name="XinT")   # X^T, later Din^T
            GiNT = persist.tile([128, N], F32, tag="GiNT", name="GiNT")
            rz_all = persist.tile([128, 2 * N], F32, tag="rz_all", name="rz_all")
            w_all = persist.tile([128, N], F32, tag="w_all", name="w_all")
            Hbuf = persist.tile([128, N + 1], F32R, tag="Hbuf", name="Hbuf")

            # chunked input loads so sweep-0 work starts after the first chunk
            for c in range(NCH):
                cs, ce = c * CH, (c + 1) * CH
                nc.sync.dma_start(out=XinT[:, cs:ce], in_=XT_d.ap()[:, cs:ce])
                nc.sync.dma_start(out=GiNT[:, cs:ce], in_=GiNT_d.ap()[:, cs:ce])

            # ---- packed per-partition constant columns [128, 32] ----
            # 4:bcomb 9-11:ebih 12-14:ebhh 15-17:dbih 18-20:dbhh
            # 21:e_brz_r 22:e_brz_z 23:d_brz_r 24:d_brz_z
            cols = gruw.tile([128, 32], F32, tag="cols", name="cols")

            def load_cols(dram_t, n_cols, at):
                src = dram_t.ap().rearrange("(g p) -> p g", p=128)
                nc.sync.dma_start(out=cols[:, at:at + n_cols], in_=src)
            load_cols(bcomb_d, 1, 4)
            load_cols(ebih_d, 3, 9)
            load_cols(ebhh_d, 3, 12)
            load_cols(dbih_d, 3, 15)
            load_cols(dbhh_d, 3, 18)
            # gate-bias ROW vectors (bih+bhh for r,z), f32r, for psum bias-matmuls
            brow_f32 = gruw.tile([1, 3 * 256], F32, tag="brow_f32", name="brow_f32")
            nc.sync.dma_start(out=brow_f32[0:1, 0:256],
                              in_=ebih_d.ap()[0:256].rearrange("(o f) -> o f", o=1))
            nc.sync.dma_start(out=brow_f32[0:1, 256:512],
                              in_=ebhh_d.ap()[0:256].rearrange("(o f) -> o f", o=1))
            nc.vector.tensor_tensor(brow_f32[0:1, 512:768], brow_f32[0:1, 0:256],
                                    brow_f32[0:1, 256:512], op=ALU.add)
            e_brz_rows = gruw.tile([1, 256], F32R, tag="e_brz", name="e_brz")
            nc.vector.tensor_copy(e_brz_rows[:], brow_f32[0:1, 512:768])
            nc.sync.dma_start(out=brow_f32[0:1, 0:256],
                              in_=dbih_d.ap()[0:256].rearrange("(o f) -> o f", o=1))
            nc.sync.dma_start(out=brow_f32[0:1, 256:512],
                              in_=dbhh_d.ap()[0:256].rearrange("(o f) -> o f", o=1))
            nc.vector.tensor_tensor(brow_f32[0:1, 512:768], brow_f32[0:1, 0:256],
                                    brow_f32[0:1, 256:512], op=ALU.add)
            d_brz_rows = gruw.tile([1, 256], F32R, tag="d_brz", name="d_brz")
            nc.vector.tensor_copy(d_brz_rows[:], brow_f32[0:1, 512:768])

            # ---- single-partition rows ----
            ones_row = gruw.tile([1, CH], F32R, tag="ones", name="ones")
            nc.sync.dma_start(out=ones_row[:], in_=ones_d.ap())
            rows = gruw.tile([1, 2 * H], F32R, tag="rows", name="rows")
            bout_sb = gruw.tile([1, 1], F32, tag="bout", name="bout")
            nc.sync.dma_start(out=rows[0:1, 0:H],
                              in_=ebhh_d.ap()[2 * H:3 * H].rearrange("(o f) -> o f", o=1).bitcast(F32R))
            nc.sync.dma_start(out=rows[0:1, H:2 * H],
                              in_=dbhh_d.ap()[2 * H:3 * H].rearrange("(o f) -> o f", o=1).bitcast(F32R))
            nc.sync.dma_start(out=bout_sb[:],
                              in_=bout_d.ap().rearrange("(o f) -> o f", o=1))

            # ---- GRU / head weights packed ----
            gw = gruw.tile([128, 4 * 3 * H + 2 * H + 1], F32R, tag="gw", name="gw")
            eWih_sb = gw[:, 0:384]
            eWhh_sb = gw[:, 384:768]
            dWih_sb = gw[:, 768:1152]
            dWhh_sb = gw[:, 1152:1536]
            Wc1_sb = gw[:, 1536:1664]
            Wc2_sb = gw[:, 1664:1792]
            Wout_sb = gw[:, 1792:1793]
            nc.sync.dma_start(out=eWih_sb, in_=eWih_d.ap())
            nc.sync.dma_start(out=eWhh_sb, in_=eWhh_d.ap())
            nc.sync.dma_start(out=dWih_sb, in_=dWih_d.ap())
            nc.sync.dma_start(out=dWhh_sb, in_=dWhh_d.ap())
            nc.sync.dma_start(out=Wc1_sb, in_=Wcomb_d.ap()[0:H, :])
            nc.sync.dma_start(out=Wc2_sb, in_=Wcomb_d.ap()[H:2 * H, :])
            nc.sync.dma_start(out=Wout_sb, in_=Wout_d.ap())

            with tc.tile_pool(name="pb_ps", bufs=2, space="PSUM") as pbps, \
                 tc.tile_pool(name="pb_tr", bufs=3) as pbtr:

                # encoder (initial h = zeros; sweep-0 Picard guess is implicit zeros)
                nc.sync.dma_start(out=Hbuf[:, 0:1], in_=zcol_d.ap())
                _gru_sweeps(nc, pbps, pbtr, ENC_SWEEPS, XinT, GiNT,
                            eWih_sb, eWhh_sb, e_brz_rows[:],
                            rows[0:1, 0:H], ones_row[:], Hbuf, rz_all, w_all,
                            zero_bias)

                # applied = E.sum(0): 8 partial reductions (overlap the last scans)
                s_part = pbtr.tile([128, 9], F32, tag="s_part", name="s_part")
                for c in range(NCH):
                    nc.vector.tensor_reduce(s_part[:, c:c + 1], Hbuf[:, c * CH + 1:(c + 1) * CH + 1],
                                            axis=mybir.AxisListType.X, op=ALU.add)
                s_col = pbtr.tile([128, 1], F32, tag="s_col", name="s_col")
                nc.vector.tensor_reduce(s_col[:], s_part[:, 0:8], axis=mybir.AxisListType.X, op=ALU.add)
                henc = pbtr.tile([128, 1], F32R, tag="henc", name="henc")
                nc.vector.tensor_copy(henc[:], Hbuf[:, N:N + 1])
                pc = pbps.tile([128, 1], F32, tag="pn", name="pc")
                nc.tensor.matmul(pc[:], Wc2_sb.bitcast(F32), s_col[:], start=True, stop=True)
                c_col = pbtr.tile([128, 1], F32, tag="c_col", name="c_col")
                nc.scalar.activation(c_col[:], pc[:], AF.Identity, bias=cols[:, 4:5], scale=1.0)

                # Din = relu(W_comb[:H]^T X + c)  (overwrites XinT), then dec GiN
                for c in range(NCH // 2):
                    cs = c * 2 * CH
                    pD = pbps.tile([128, 2 * CH], F32, tag="pr", name="pD")
                    nc.tensor.matmul(pD[:, 0:CH], Wc1_sb, XinT[:, cs:cs + CH], start=True, stop=True)
                    nc.tensor.matmul(pD[:, CH:2 * CH], Wc1_sb, XinT[:, cs + CH:cs + 2 * CH],
                                     start=True, stop=True)
                    nc.scalar.activation(XinT[:, cs:cs + 2 * CH], pD[:], AF.Relu,
                                         bias=c_col[:, 0:1], scale=1.0)
                for c in range(NCH // 2):
                    cs = c * 2 * CH
                    pG = pbps.tile([128, 2 * CH], F32, tag="pr", name="pG2")
                    nc.tensor.matmul(pG[:, 0:CH], dWih_sb[:, 2 * H:3 * H], XinT[:, cs:cs + CH],
                                     start=True, stop=True)
                    nc.tensor.matmul(pG[:, CH:2 * CH], dWih_sb[:, 2 * H:3 * H],
                                     XinT[:, cs + CH:cs + 2 * CH], start=True, stop=True)
                    nc.scalar.activation(GiNT[:, cs:cs + 2 * CH], pG[:], AF.Identity,
                                         bias=cols[:, 17:18], scale=1.0)

                # decoder (initial h = h_enc; sweep-0 Picard guess is implicit zeros)
                nc.vector.tensor_copy(Hbuf[:, 0:1], henc[:])
                _gru_sweeps(nc, pbps, pbtr, DEC_SWEEPS, XinT, GiNT,
                            dWih_sb, dWhh_sb, d_brz_rows[:],
                            rows[0:1, H:2 * H], ones_row[:], Hbuf, rz_all, w_all,
                            zero_bias)

                # output = sigmoid(D @ W_out + b_out), chunk pairs, streamed out
                for c in range(NCH // 2):
                    cs = c * 2 * CH
                    po = pbps.tile([1, 2 * CH], F32, tag="pn", name="po")
                    nc.tensor.matmul(po[:, 0:CH], Wout_sb, Hbuf[:, cs + 1:cs + CH + 1],
                                     start=True, stop=True)
                    nc.tensor.matmul(po[:, CH:2 * CH], Wout_sb, Hbuf[:, cs + CH + 1:cs + 2 * CH + 1],
                                     start=True, stop=True)
                    o_t = pbtr.tile([1, 2 * CH], F32, tag="o_t", name="o_t")
                    nc.scalar.activation(o_t[:], po[:], AF.Sigmoid,
                                         bias=bout_sb[0:1, 0:1], scale=1.0)
                    nc.sync.dma_start(out=out_d.ap()[cs:cs + 2 * CH].rearrange("(o n) -> o n", o=1),
                                      in_=o_t[:])

    nc.compile()
    return nc


_NC_A = None
_NC_B = {}


def _get_programs(zero_bias):
    global _NC_A
    if _NC_A is None:
        _NC_A = build_program_a()
    if zero_bias not in _NC_B:
        _NC_B[zero_bias] = build_program_b(zero_bias)
    return _NC_A, _NC_B[zero_bias]


_LAST_EXEC_NS = None


def kernel(**inputs):
    global _LAST_EXEC_NS
    zero_bias = not any(
        np.any(np.asarray(inputs[k]) != 0)
        for k in ("enc_bih", "enc_bhh", "dec_bih", "dec_bhh"))
    nc_a, nc_b = _get_programs(zero_bias)
    f32c = lambda a: np.ascontiguousarray(np.asarray(a), dtype=np.float32)

    bfT = f32c(np.asarray(inputs["boxes_feature"])[0].T)   # [1024, N]
    bsT = f32c(np.asarray(inputs["boxes_box_score"])[0].T)  # [2560, N]
    bbT = f32c(np.asarray(inputs["boxes_box"])[0].T)        # [320, N]
    weights_a = {
        "W_appear": f32c(inputs["W_appear"]), "b_appear": f32c(inputs["b_appear"]),
        "W_s1": f32c(inputs["W_s1"]), "b_s1": f32c(inputs["b_s1"]),
        "W_s2": f32c(inputs["W_s2"]), "b_s2": f32c(inputs["b_s2"]),
        "W_box": f32c(inputs["W_box"]), "b_box": f32c(inputs["b_box"]),
        "W_all": f32c(inputs["W_all"]), "b_all": f32c(inputs["b_all"]),
        "enc_Wih_n": f32c(np.asarray(inputs["enc_Wih"])[:, 2 * H:3 * H]),
        "enc_bih_n": f32c(np.asarray(inputs["enc_bih"])[2 * H:3 * H]),
    }
    in_maps_a = []
    for c in range(NCORES):
        cs, ce = c * CH, (c + 1) * CH
        in_maps_a.append({
            "bfT": f32c(bfT[:, cs:ce]),
            "bsT": f32c(bsT[:, cs:ce]),
            "bbT": f32c(bbT[:, cs:ce]),
            **weights_a,
        })
    res_a = run_bass_kernel_spmd(nc_a, in_maps_a, list(range(NCORES)))
    XT = np.concatenate([np.asarray(r["xt_out"]) for r in res_a.results], axis=1)
    GiNT = np.concatenate([np.asarray(r["gin_out"]) for r in res_a.results], axis=1)

    in_map_b = {
        "XT": np.ascontiguousarray(XT), "GiNT": np.ascontiguousarray(GiNT),
        "enc_Wih": f32c(inputs["enc_Wih"]), "enc_Whh": f32c(inputs["enc_Whh"]),
        "enc_bih": f32c(inputs["enc_bih"]), "enc_bhh": f32c(inputs["enc_bhh"]),
        "dec_Wih": f32c(inputs["dec_Wih"]), "dec_Whh": f32c(inputs["dec_Whh"]),
        "dec_bih": f32c(inputs["dec_bih"]), "dec_bhh": f32c(inputs["dec_bhh"]),
        "W_comb": f32c(inputs["W_comb"]), "b_comb": f32c(inputs["b_comb"]),
        "W_out": f32c(inputs["W_out"]), "b_out": f32c(inputs["b_out"]),
        "ones_row": np.ones((1, CH), np.float32),
        "zeros_col": np.zeros((128, 1), np.float32),
    }
    res_b = run_bass_kernel_spmd(nc_b, [in_map_b], [0])
    out = np.asarray(res_b.results[0]["out"], dtype=np.float32).reshape(N, 1)
    labels = np.asarray(inputs["boxes_label"], dtype=np.float32)[0, :N]
    weights = np.asarray(inputs["boxes_weight"], dtype=np.float32)[0, :N]
    return out, labels, weights
